# revision 65
# baseline (speedup 1.0000x reference)
"""GRU-decoder first-step kernel for 8 Trainium2 NeuronCores.

Math (see reference): all L-1 output steps are identical, so compute the
single step and broadcast on host:
    x0 = relu(emb[input_ids[:, 0]])                [B, D]
    h0 = einsum("bld,l->bd", hidden, Wb[0]) + bb   [B, D]
    GRU cell (r, z, n) -> h1                       [B, D]
    logits = h1 @ Wp.T + bp                        [B, V]
    out = broadcast(logits)                        [B, L-1, V]

Sharding: tensor-parallel over the vocab dim for Wp (the only big
tensor); the tiny GRU math is replicated on every core.

Memory-bound regime -> built around minimizing HBM bytes and keeping
the (shared, arrival-order) DMA engines streaming gaplessly. Perf on
the concourse TimelineSim cost model: 23.5 us/core vs 40.5 us for the
fp16 baseline (pure-transfer floor for the 6.0 MB each core now reads
is ~16.7 us; the rest is the ~1.9 us pre-stream issue latency and a
~4.9 us tail of fixed DMA/sem/drain costs behind the last wp piece).
Key decisions, in impact order:
 - fp8 e3m4 (pow2-scaled; PSUM still accumulates fp32) for every
   weight matrix: Wp and W_ih/W_hh x128, hidden x2. e3m4's 4 mantissa
   bits halve e4m3's RMS error for these Gaussian tensors; measured
   output error is 1.50e-2 scale-relative against the 2e-2 gate
   (e4m3 measures 2.2e-2 - fails). Descales are free: h1 ships to
   the projection as h1/128 (exact pow2 in fp16), gate preacts and h0
   descale inside the activation's scale multiplier. wbd/x0/bias
   stay fp16 - the h0 path hits the logits ~1:1 (h1 ~= z*h0 + n/2),
   so 1%-level noise there would eat the remaining error budget.
 - projection runs TRANSPOSED: per 128-row vocab block, stationary
   lhsT = wp tile [128d, 128v] (fp8), moving rhs = h1T [128d, 32b]
   (fp16) -> PSUM [128v, 32b]; bp rides a K=1 matmul per block. The
   vocab shard is exactly ceil(V/8)=6283 rows (49 full blocks + one
   ragged 11-row block) - no padded rows ride the stream. PSUM banks
   map one-to-one onto flush groups: a start pending-zeroes the whole
   2KB bank region (one open accumulation group per bank at a time),
   and sharing a bank between a flush's copy and later blocks' mms
   creates a bank-granular WAR that gates the tail - so tail blocks
   44-49 own bank 3 outright while flush (32,44) reads bank 2.
 - logits write back as fp16 in the blocked [128, blk*B] layout (host
   de-interleaves + upcasts), halving output bytes.
 - load order = dependency order on ONE stream: hidden (bridge) ->
   gate weights (r, n, z consumption order) -> wp pieces. The DMA
   device serves transfers in arrival order, so anything h1 needs
   must physically precede wp; an earlier revision that put gate
   loads on the gpsimd queue saw them arbitrate behind wp and the
   whole projection serialize after the stream (+8 us).
 - tail: the last wp piece covers exactly the last flush (one
   dependency class - the tile scheduler otherwise shuffles tail
   matmuls), its two PSUM-bank copies split across ACT and DVE, and
   the final out-DMA rides the idle sync queue, the shortest fixed
   post-wait DGE path. Early flushes go out via gpsimd mid-stream.
 - a prepared-SWDGE scatter + trigger_dma tail (to skip the ~2.1 us
   post-wait HWDGE pipeline) was tried and REVERTED: the NEFF-level
   runtime does not honor the prep/trigger deferred-read ordering
   (scatters fired before their source copies; TimelineSim also
   deadlocks on the orphaned DMASW lane sem).
"""

import numpy as np
import ml_dtypes

import concourse.mybir as mybir
import concourse.tile as tile
from concourse import bacc
from concourse.bass_utils import run_bass_kernel_spmd

B, L, D, V = 32, 64, 512, 50257
NCORES = 8
VS = 6283           # per-core vocab shard; 8 * 6283 = 50264 >= V
VPAD = VS * NCORES
NBLK = 50           # 49 full 128-row vocab blocks + one ragged 11-row
LB = VS - 49 * 128  # ragged last block rows (11)
DC = D // 128       # 4 contraction chunks of 128
NT = (B * L) // 128  # 16 hidden row-tiles of 128
G = 3 * D
GW = DC * 512


def _bw(b):
    """wp column offset of block b (blocks are [k, j] interleaved)."""
    return b * 512 if b < 49 else 49 * 512 + (b - 49) * DC * LB
F16 = mybir.dt.float16
F32 = mybir.dt.float32
F8 = mybir.dt.float8e3
E3M4 = ml_dtypes.float8_e3m4

# pow2 quantization scales (max |w| ~0.1 << e3m4 max 15.5 at x128;
# hidden absmax ~5 -> x2)
SP = 128.0   # Wp scale; h1 ships as h1/SP (exact pow2 in fp16)
SG = 128.0   # gate weight scale; descaled in the activation multiplier
SH = 2.0     # hidden scale; descaled in the h0 copies' activation scale

# wp block-piece split: sizes shrink toward the end so the last-arriving
# piece carries minimal compute before the final flush. The last piece
# matches the last flush exactly - one dependency class, so the tile
# scheduler cannot shuffle tail matmuls into a worse order.
WP_PIECES = [(0, 10), (10, 20), (20, 30), (30, 40), (40, 44), (44, 50)]
# logits flush groups (block ranges); the tail is ONE flush class with
# ONE piece and a private PSUM bank - splitting it (by piece or flush)
# measurably regresses: the tile scheduler reorders multi-class tail
# matmuls so everything gates on the last semaphore anyway
FLUSHES = [(0, 16), (16, 32), (32, 44), (44, 50)]

# smalls row layout: [bp | bcat(=b_ih,b_hh scaled SG) | bb_row]
BP_OFF = 0
BCAT_OFF = VS
BB_OFF = VS + 2 * G
NSMALL = VS + 2 * G + B

_CACHE: dict = {}


def _build_bass():
    nc = bacc.Bacc("TRN2", target_bir_lowering=False, debug=False,
                   num_devices=NCORES)

    # wp columns: [blk, k, j] so each 512-col block window holds all DC
    # contraction strips for one vocab block
    wp_d = nc.dram_tensor("wpT", [128, DC * VS], F8, kind="ExternalInput")
    # head = [wb compact (NT*2) | x0T (DC*B)] columns fp16; hidden rides
    # its own fp8 tensor (x SH). The bridge weight column pair for tile t
    # is (Wb in partitions 0:64, 0) and (0, Wb in partitions 64:128) -
    # the block-diagonal structure collapses to 2 columns per tile.
    NHEAD = NT * 2 + DC * B
    head_d = nc.dram_tensor("head", [128, NHEAD], F16, kind="ExternalInput")
    hid_d = nc.dram_tensor("hid8", [128, NT * D], F8, kind="ExternalInput")
    wih_d = nc.dram_tensor("wihT", [128, DC * G], F8, kind="ExternalInput")
    whh_d = nc.dram_tensor("whhT", [128, DC * G], F8, kind="ExternalInput")
    smalls_d = nc.dram_tensor("smalls", [1, NSMALL], F16, kind="ExternalInput")
    out_d = nc.dram_tensor("logits", [128, NBLK * B], F16,
                           kind="ExternalOutput")

    AF = mybir.ActivationFunctionType

    with tile.TileContext(nc) as tc:
        with (
            tc.tile_pool(name="wp", bufs=1) as wp_pool,
            tc.tile_pool(name="big", bufs=1) as big,
            tc.tile_pool(name="sm", bufs=1) as sm,
            tc.tile_pool(name="ps_p", bufs=1, space="PSUM") as ps_p,
            tc.tile_pool(name="ps_g", bufs=3, space="PSUM") as ps_g,
            tc.tile_pool(name="ps_b", bufs=1, space="PSUM") as ps_b,
        ):
            # ---- loads. The gpsimd SWDGE queue opens the stream (its
            # first transfer reaches the DMA device ~0.2us before the sync
            # path) carrying hidden + head + smalls; the sync queue streams
            # the gate weights and wp pieces behind it. ----
            hid_sb = big.tile([128, NT * D], F8, tag="hid")
            nc.gpsimd.dma_start(out=hid_sb[:], in_=hid_d[:])
            head_sb = big.tile([128, NHEAD], F16, tag="head")
            nc.gpsimd.dma_start(out=head_sb[:], in_=head_d[:])
            wbd_sb = head_sb[:, 0:NT * 2]
            x0f_sb = head_sb[:, NT * 2:]

            smalls_sb = sm.tile([1, NSMALL], F16, tag="smalls")
            nc.gpsimd.dma_start(out=smalls_sb[:], in_=smalls_d[:])
            # W columns are laid out [gate, k, j_local] on host; load in
            # consumption order r, n, z so the z load overlaps the r/n
            # math. Gate loads stay on the sync queue between head and wp:
            # the DMA device serves transfers in arrival order, so a slow
            # queue would push the z gate (and with it h1 and the entire
            # projection) to the end of the stream.
            wih_sb = big.tile([128, DC * G], F8, tag="wih")
            whh_sb = big.tile([128, DC * G], F8, tag="whh")
            for g_ in (0, 2, 1):  # r, n, z
                nc.sync.dma_start(out=wih_sb[:, g_ * GW:(g_ + 1) * GW],
                                  in_=wih_d[:, g_ * GW:(g_ + 1) * GW])
                nc.sync.dma_start(out=whh_sb[:, g_ * GW:(g_ + 1) * GW],
                                  in_=whh_d[:, g_ * GW:(g_ + 1) * GW])

            wp_sb = wp_pool.tile([128, DC * VS], F8, tag="wp")
            for b0, b1 in WP_PIECES:
                nc.sync.dma_start(out=wp_sb[:, _bw(b0):_bw(b1)],
                                  in_=wp_d[:, _bw(b0):_bw(b1)])

            logit_sb = big.tile([128, NBLK * B], F16, tag="lg")

            ones_sb = sm.tile([1, 128], F16, tag="ones")
            nc.any.memset(ones_sb[:], 1.0)

            # ---- projection PSUM: 4 banks x 16 block slots of [128, B].
            # One accumulation group per bank may be open at a time (a
            # start pending-zeroes the whole 2KB bank region), so each
            # block's group = bias mm + DC wp mms, sequential per bank;
            # closed blocks' values persist until their flush copy. ----
            ps_tiles = [ps_p.tile([128, 16 * B], F32, tag=f"p{i}",
                                  name=f"pst{i}")
                        for i in range(len(FLUSHES))]

            # one PSUM bank per flush group: tail blocks must not share a
            # bank with an earlier flush's blocks, or the bank-granular
            # WAR against that flush's PSUM->SBUF copy gates the tail mms
            def blk_ps(blk):
                fi = next(i for i, (b0, b1) in enumerate(FLUSHES)
                          if b0 <= blk < b1)
                o = (blk - FLUSHES[fi][0]) * B
                return ps_tiles[fi][:, o:o + B]

            # ==== GRU cell entirely in transposed space: all [d, b]
            # tensors are [128, DC*B] tiles with column = k*B + b. ====

            # x0T = relu(embT rows) -> fp16, one op
            x0t_sb = sm.tile([128, DC * B], F16, tag="x0t")
            nc.scalar.activation(x0t_sb[:], x0f_sb, AF.Relu)

            # bridge, transposed: h0T[d, b] = sum_t hid[t].T @ wbd[t] + bb.
            # hid is fp8 x SH; bb rides a K=1 matmul with a host-side x SH
            # (ones along M, bb_row along N); the copies descale by 1/SH.
            ps_h0 = ps_b.tile([128, DC * B], F32, tag="b")
            for k in range(DC):
                o = ps_h0[:, k * B:(k + 1) * B]
                nc.tensor.matmul(o, ones_sb[:, 0:128],
                                 smalls_sb[:, BB_OFF:BB_OFF + B],
                                 start=True, stop=False)
                for t_i in range(NT):
                    nc.tensor.matmul(
                        o[:, 2 * t_i:2 * t_i + 2],
                        hid_sb[:, t_i * D + k * 128:t_i * D + (k + 1) * 128],
                        wbd_sb[:, t_i * 2:(t_i + 1) * 2],
                        start=False, stop=(t_i == NT - 1),
                    )
            h0t_sb = sm.tile([128, DC * B], F32, tag="h0t")
            nc.scalar.activation(h0t_sb[:], ps_h0[:], AF.Copy, scale=1.0 / SH)
            h0t16 = sm.tile([128, DC * B], F16, tag="h0t16")
            nc.scalar.activation(h0t16[:], ps_h0[:], AF.Copy, scale=1.0 / SH)

            # gates, transposed: gate^T[j, b] accumulated per
            # (j-block jb, k): lhsT = W^T [d-chunk, j-block] (fp8 e3m4,
            # x SG), rhs = x0T / h0T [d-chunk, b] (fp16, true scale).
            # Bias rows (x SG on host) ride K=1 matmuls with values
            # along M; the 1/SG descale hides in the activation scale.
            def gate_psum(g_, use_x, use_h):
                ps = ps_g.tile([128, DC * B], F32, tag="g")
                for jb in range(DC):
                    o = ps[:, jb * B:(jb + 1) * B]
                    bw = BCAT_OFF + g_ * 512 + jb * 128
                    ops = []
                    if use_x:
                        ops.append((smalls_sb[:, bw:bw + 128],
                                    ones_sb[:, 0:B]))
                    if use_h:
                        ops.append((smalls_sb[:, G + bw:G + bw + 128],
                                    ones_sb[:, 0:B]))
                    for k in range(DC):
                        cw = g_ * GW + k * 512 + jb * 128
                        if use_x:
                            ops.append((wih_sb[:, cw:cw + 128],
                                        x0t_sb[:, k * B:(k + 1) * B]))
                        if use_h:
                            ops.append((whh_sb[:, cw:cw + 128],
                                        h0t16[:, k * B:(k + 1) * B]))
                    for i, (lhsT, rhs) in enumerate(ops):
                        nc.tensor.matmul(o, lhsT, rhs, start=(i == 0),
                                         stop=(i == len(ops) - 1))
                return ps

            ps_r = gate_psum(0, True, True)
            ps_xn = gate_psum(2, True, False)
            ps_hn = gate_psum(2, False, True)

            r_sb = sm.tile([128, DC * B], F32, tag="r")
            nc.scalar.activation(r_sb[:], ps_r[:], AF.Sigmoid, scale=1.0 / SG)
            t1 = sm.tile([128, DC * B], F32, tag="t1")
            nc.vector.tensor_mul(t1[:], r_sb[:], ps_hn[:])
            t2 = sm.tile([128, DC * B], F32, tag="t2")
            nc.vector.tensor_add(t2[:], t1[:], ps_xn[:])
            n_sb = sm.tile([128, DC * B], F32, tag="n")
            nc.scalar.activation(n_sb[:], t2[:], AF.Tanh, scale=1.0 / SG)

            ps_z = gate_psum(1, True, True)
            z_sb = sm.tile([128, DC * B], F32, tag="z")
            nc.scalar.activation(z_sb[:], ps_z[:], AF.Sigmoid, scale=1.0 / SG)

            # h1T = n + z * (h0T - n); ship to the projection as
            # h1/SP in fp16 (exact pow2 scaling)
            d_sb = sm.tile([128, DC * B], F32, tag="d")
            nc.vector.tensor_sub(d_sb[:], h0t_sb[:], n_sb[:])
            e_sb = sm.tile([128, DC * B], F32, tag="e")
            nc.vector.tensor_mul(e_sb[:], z_sb[:], d_sb[:])
            h1_sb = sm.tile([128, DC * B], F32, tag="h1")
            nc.vector.tensor_add(h1_sb[:], n_sb[:], e_sb[:])
            h1t_sb = sm.tile([128, DC * B], F16, tag="h1t")
            nc.scalar.activation(h1t_sb[:], h1_sb[:], AF.Copy, scale=1.0 / SP)

            # ---- projection: PSUM[blk][j, b] (+= bp already) +=
            # sum_k wp[blk,k].T @ h1T[k] ----
            flush_at = {b1 - 1: i for i, (b0, b1) in enumerate(FLUSHES)}
            # out-DMA queue per flush: early ones ride gpsimd (Pool idle,
            # off critical path), the tail ones the idle sync queue (the
            # shortest fixed post-wait DGE path of the HWDGE engines)
            F_DMA = [nc.gpsimd, nc.gpsimd, nc.sync, nc.sync]
            for blk in range(NBLK):
                nb = 128 if blk < 49 else LB  # ragged last block
                win = _bw(blk)
                ps = blk_ps(blk)[0:nb, :]
                nc.tensor.matmul(
                    ps,
                    smalls_sb[:, BP_OFF + blk * 128:BP_OFF + blk * 128 + nb],
                    ones_sb[:, 0:B], start=True, stop=False)
                for k in range(DC):
                    nc.tensor.matmul(
                        ps, wp_sb[:, win + k * nb:win + (k + 1) * nb],
                        h1t_sb[:, k * B:(k + 1) * B],
                        start=False, stop=(k == DC - 1))
                if blk in flush_at:
                    fi = flush_at[blk]
                    b0, b1 = FLUSHES[fi]
                    lo, hi = b0 * B, b1 * B
                    nc.scalar.activation(
                        logit_sb[:, lo:hi],
                        ps_tiles[fi][:, 0:(b1 - b0) * B], AF.Copy)
                    F_DMA[fi].dma_start(out=out_d[:, lo:hi],
                                        in_=logit_sb[:, lo:hi])

    nc.compile()
    return nc


def _get_bass():
    if "nc" not in _CACHE:
        _CACHE["nc"] = _build_bass()
    return _CACHE["nc"]


def _interleave(a):
    """[DC*128, N] -> [128, DC*N] with [p, k*N+c] = a[k*128+p, c]."""
    n = a.shape[1]
    return np.ascontiguousarray(
        a.reshape(DC, 128, n).transpose(1, 0, 2).reshape(128, DC * n))


def _prep_inputs(inputs):
    def _e3m4(a):
        # saturate instead of overflowing to inf if a weight ever exceeds
        # the (amply margined) hardcoded pow2 scales
        return np.clip(a, -15.5, 15.5).astype(E3M4)

    ids = np.asarray(inputs["input_ids"])[:, 0].astype(np.int64)
    emb = np.asarray(inputs["emb"], dtype=np.float32)
    hidden = np.asarray(inputs["hidden"], dtype=np.float32)
    Wb = np.asarray(inputs["Wb"], dtype=np.float32)
    bb = np.asarray(inputs["bb"], dtype=np.float32)
    W_ih = np.asarray(inputs["W_ih"], dtype=np.float32)
    b_ih = np.asarray(inputs["b_ih"], dtype=np.float32)
    W_hh = np.asarray(inputs["W_hh"], dtype=np.float32)
    b_hh = np.asarray(inputs["b_hh"], dtype=np.float32)
    Wp = np.asarray(inputs["Wp"], dtype=np.float32)
    bp = np.asarray(inputs["bp"], dtype=np.float32)

    # x0T: [D, B] -> [128, DC*B]
    x0T = _interleave(np.ascontiguousarray(emb[ids].T))
    # hidden rows (b*64+l, d) -> [128, NT*D] with [p, t*D+d] = row t*128+p
    hid8 = _e3m4(np.ascontiguousarray(
        hidden.reshape(NT, 128, D).transpose(1, 0, 2)
        .reshape(128, NT * D) * SH))
    # compact block-diagonal bridge weights: 2 columns per row-tile
    wbd = np.zeros((128, NT * 2), np.float32)
    wbd[0:64, 0::2] = Wb[0][:, None]
    wbd[64:128, 1::2] = Wb[0][:, None]
    head = np.concatenate(
        [wbd.astype(np.float16), x0T.astype(np.float16)], axis=1)

    def _w_layout(w):
        # W [3D, D] -> cols [g, k, j_local], partitions d%128; fp8 x SG
        a = np.ascontiguousarray(w.T * SG).reshape(DC, 128, 3, 512)
        return _e3m4(np.ascontiguousarray(
            a.transpose(1, 2, 0, 3).reshape(128, DC * G)))

    wihT = _w_layout(W_ih)
    whhT = _w_layout(W_hh)

    smalls = np.zeros((1, NSMALL), np.float16)
    smalls[0, BCAT_OFF:BCAT_OFF + 2 * G] = \
        np.concatenate([b_ih, b_hh]) * SG
    smalls[0, BB_OFF:BB_OFF + B] = bb[0] * SH

    # Wp rows padded to VPAD, scaled, fp8; per-core layout
    # wp8[p, blk*512 + k*128 + j] = Wp[c*VS + blk*128 + j, k*128 + p] * SP
    Wp_pad = np.zeros((VPAD, D), np.float32)
    Wp_pad[:V] = Wp * SP
    bp_pad = np.zeros((VPAD,), np.float16)
    bp_pad[:V] = bp

    shared = {"head": head, "hid8": hid8, "wihT": wihT, "whhT": whhT}
    in_maps = []
    for c in range(NCORES):
        m = dict(shared)
        shard = Wp_pad[c * VS:(c + 1) * VS]
        full = shard[:49 * 128].reshape(49, 128, DC, 128) \
            .transpose(3, 0, 2, 1).reshape(128, 49 * 512)
        rag = shard[49 * 128:].reshape(LB, DC, 128) \
            .transpose(2, 1, 0).reshape(128, DC * LB)
        m["wpT"] = _e3m4(np.ascontiguousarray(
            np.concatenate([full, rag], axis=1)))
        sm_c = smalls.copy()
        sm_c[0, BP_OFF:BP_OFF + VS] = bp_pad[c * VS:(c + 1) * VS]
        m["smalls"] = sm_c
        in_maps.append(m)
    return in_maps


def _run(in_maps, trace=False, tmpdir=None):
    nc = _get_bass()
    return run_bass_kernel_spmd(nc, in_maps, list(range(NCORES)),
                                trace=trace, tmpdir=tmpdir)


def kernel(**inputs) -> np.ndarray:
    in_maps = _prep_inputs(inputs)
    try:
        res = _run(in_maps).results
    except Exception:
        # transient NRT device wedges have been observed on this fabric;
        # one retry after a short pause usually lands on healthy cores
        import time as _time
        _time.sleep(5.0)
        res = _run(in_maps).results
    # de-interleave: out[p, blk*B + b] = logits[c*VS + blk*128 + p, b]
    parts = []
    for c in range(NCORES):
        a = np.asarray(res[c]["logits"]).astype(np.float32)
        parts.append(a.reshape(128, NBLK, B).transpose(1, 0, 2)
                     .reshape(NBLK * 128, B)[:VS])
    logits = np.concatenate(parts, axis=0)[:V].T
    logits = np.ascontiguousarray(logits)
    return np.broadcast_to(logits[:, None, :], (B, L - 1, V))


# revision 66
# speedup vs baseline: 1.0043x; 1.0043x over previous
"""GRU-decoder first-step kernel for 8 Trainium2 NeuronCores.

Math (see reference): all L-1 output steps are identical, so compute the
single step and broadcast on host:
    x0 = relu(emb[input_ids[:, 0]])                [B, D]
    h0 = einsum("bld,l->bd", hidden, Wb[0]) + bb   [B, D]
    GRU cell (r, z, n) -> h1                       [B, D]
    logits = h1 @ Wp.T + bp                        [B, V]
    out = broadcast(logits)                        [B, L-1, V]

Sharding: tensor-parallel over the vocab dim for Wp (the only big
tensor); the tiny GRU math is replicated on every core.

Memory-bound regime -> built around minimizing HBM bytes and keeping
the (shared, arrival-order) DMA engines streaming gaplessly. Perf on
the concourse TimelineSim cost model: 23.5 us/core vs 40.5 us for the
fp16 baseline (pure-transfer floor for the 6.0 MB each core now reads
is ~16.7 us; the rest is the ~1.9 us pre-stream issue latency and a
~4.9 us tail of fixed DMA/sem/drain costs behind the last wp piece).
Key decisions, in impact order:
 - fp8 e3m4 (pow2-scaled; PSUM still accumulates fp32) for every
   weight matrix: Wp and W_ih/W_hh x128, hidden x2. e3m4's 4 mantissa
   bits halve e4m3's RMS error for these Gaussian tensors; measured
   output error is 1.50e-2 scale-relative against the 2e-2 gate
   (e4m3 measures 2.2e-2 - fails). Descales are free: h1 ships to
   the projection as h1/128 (exact pow2 in fp16), gate preacts and h0
   descale inside the activation's scale multiplier. wbd/x0/bias
   stay fp16 - the h0 path hits the logits ~1:1 (h1 ~= z*h0 + n/2),
   so 1%-level noise there would eat the remaining error budget.
 - projection runs TRANSPOSED: per 128-row vocab block, stationary
   lhsT = wp tile [128d, 128v] (fp8), moving rhs = h1T [128d, 32b]
   (fp16) -> PSUM [128v, 32b]; bp rides a K=1 matmul per block. The
   vocab shard is exactly ceil(V/8)=6283 rows (49 full blocks + one
   ragged 11-row block) - no padded rows ride the stream. PSUM banks
   map one-to-one onto flush groups: a start pending-zeroes the whole
   2KB bank region (one open accumulation group per bank at a time),
   and sharing a bank between a flush's copy and later blocks' mms
   creates a bank-granular WAR that gates the tail - so tail blocks
   44-49 own bank 3 outright while flush (32,44) reads bank 2.
 - logits write back as fp16 in the blocked [128, blk*B] layout (host
   de-interleaves + upcasts), halving output bytes.
 - load order = dependency order on ONE stream: hidden (bridge) ->
   gate weights (r, n, z consumption order) -> wp pieces. The DMA
   device serves transfers in arrival order, so anything h1 needs
   must physically precede wp; an earlier revision that put gate
   loads on the gpsimd queue saw them arbitrate behind wp and the
   whole projection serialize after the stream (+8 us).
 - tail: the last wp piece covers exactly the last flush (one
   dependency class - the tile scheduler otherwise shuffles tail
   matmuls), its two PSUM-bank copies split across ACT and DVE, and
   the final out-DMA rides the idle sync queue, the shortest fixed
   post-wait DGE path. Early flushes go out via gpsimd mid-stream.
 - a prepared-SWDGE scatter + trigger_dma tail (to skip the ~2.1 us
   post-wait HWDGE pipeline) was tried and REVERTED: the NEFF-level
   runtime does not honor the prep/trigger deferred-read ordering
   (scatters fired before their source copies; TimelineSim also
   deadlocks on the orphaned DMASW lane sem).
"""

import numpy as np
import ml_dtypes

import concourse.mybir as mybir
import concourse.tile as tile
from concourse import bacc
from concourse.bass_utils import run_bass_kernel_spmd

B, L, D, V = 32, 64, 512, 50257
NCORES = 8
VS = 6283           # per-core vocab shard; 8 * 6283 = 50264 >= V
VPAD = VS * NCORES
NBLK = 50           # 49 full 128-row vocab blocks + one ragged 11-row
LB = VS - 49 * 128  # ragged last block rows (11)
DC = D // 128       # 4 contraction chunks of 128
NT = (B * L) // 128  # 16 hidden row-tiles of 128
G = 3 * D
GW = DC * 512


def _bw(b):
    """wp column offset of block b (blocks are [k, j] interleaved)."""
    return b * 512 if b < 49 else 49 * 512 + (b - 49) * DC * LB
F16 = mybir.dt.float16
F32 = mybir.dt.float32
F8 = mybir.dt.float8e3
E3M4 = ml_dtypes.float8_e3m4

# pow2 quantization scales (max |w| ~0.1 << e3m4 max 15.5 at x128;
# hidden absmax ~5 -> x2)
SP = 128.0   # Wp scale; h1 ships as h1/SP (exact pow2 in fp16)
SG = 128.0   # gate weight scale; descaled in the activation multiplier
SH = 2.0     # hidden scale; descaled in the h0 copies' activation scale

# wp block-piece split: sizes shrink toward the end so the last-arriving
# piece carries minimal compute before the final flush. The last piece
# matches the last flush exactly - one dependency class, so the tile
# scheduler cannot shuffle tail matmuls into a worse order.
WP_PIECES = [(0, 10), (10, 20), (20, 30), (30, 40), (40, 44), (44, 50)]
# logits flush groups (block ranges); the tail is ONE flush class with
# ONE piece and a private PSUM bank - splitting it (by piece or flush)
# measurably regresses: the tile scheduler reorders multi-class tail
# matmuls so everything gates on the last semaphore anyway
FLUSHES = [(0, 16), (16, 32), (32, 44), (44, 50)]

# smalls row layout: [bp | bcat(=b_ih,b_hh scaled SG) | bb_row]
BP_OFF = 0
BCAT_OFF = VS
BB_OFF = VS + 2 * G
NSMALL = VS + 2 * G + B

_CACHE: dict = {}


def _build_bass():
    nc = bacc.Bacc("TRN2", target_bir_lowering=False, debug=False,
                   num_devices=NCORES)

    # wp columns: [blk, k, j] so each 512-col block window holds all DC
    # contraction strips for one vocab block
    wp_d = nc.dram_tensor("wpT", [128, DC * VS], F8, kind="ExternalInput")
    # head = [wb compact (NT*2) | x0T (DC*B)] columns fp16; hidden rides
    # its own fp8 tensor (x SH). The bridge weight column pair for tile t
    # is (Wb in partitions 0:64, 0) and (0, Wb in partitions 64:128) -
    # the block-diagonal structure collapses to 2 columns per tile.
    NHEAD = NT * 2 + DC * B
    head_d = nc.dram_tensor("head", [128, NHEAD], F16, kind="ExternalInput")
    hid_d = nc.dram_tensor("hid8", [128, NT * D], F8, kind="ExternalInput")
    wih_d = nc.dram_tensor("wihT", [128, DC * G], F8, kind="ExternalInput")
    whh_d = nc.dram_tensor("whhT", [128, DC * G], F8, kind="ExternalInput")
    smalls_d = nc.dram_tensor("smalls", [1, NSMALL], F16, kind="ExternalInput")
    out_d = nc.dram_tensor("logits", [128, NBLK * B], F16,
                           kind="ExternalOutput")

    AF = mybir.ActivationFunctionType

    with tile.TileContext(nc) as tc:
        with (
            tc.tile_pool(name="wp", bufs=1) as wp_pool,
            tc.tile_pool(name="big", bufs=1) as big,
            tc.tile_pool(name="sm", bufs=1) as sm,
            tc.tile_pool(name="ps_p", bufs=1, space="PSUM") as ps_p,
            tc.tile_pool(name="ps_g", bufs=3, space="PSUM") as ps_g,
            tc.tile_pool(name="ps_b", bufs=1, space="PSUM") as ps_b,
        ):
            # ---- loads. Big reads stream on the sync (SP) queue in
            # critical-chain order; the tiny head + smalls ride the gpsimd
            # SWDGE queue (issue cost on the otherwise-idle Pool engine).
            # Leading the stream with a Pool-issued DMA (nominally ~0.2us
            # faster to first byte) measures WORSE every time - don't. ----
            head_sb = big.tile([128, NHEAD], F16, tag="head")
            nc.gpsimd.dma_start(out=head_sb[:], in_=head_d[:])
            wbd_sb = head_sb[:, 0:NT * 2]
            x0f_sb = head_sb[:, NT * 2:]
            hid_sb = big.tile([128, NT * D], F8, tag="hid")
            HHALF = (NT // 2) * D
            nc.sync.dma_start(out=hid_sb[:, 0:HHALF], in_=hid_d[:, 0:HHALF])
            nc.sync.dma_start(out=hid_sb[:, HHALF:], in_=hid_d[:, HHALF:])

            smalls_sb = sm.tile([1, NSMALL], F16, tag="smalls")
            nc.gpsimd.dma_start(out=smalls_sb[:], in_=smalls_d[:])
            # W columns are laid out [gate, k, j_local] on host; load in
            # consumption order r, n, z so the z load overlaps the r/n
            # math. Gate loads stay on the sync queue between head and wp:
            # the DMA device serves transfers in arrival order, so a slow
            # queue would push the z gate (and with it h1 and the entire
            # projection) to the end of the stream.
            wih_sb = big.tile([128, DC * G], F8, tag="wih")
            whh_sb = big.tile([128, DC * G], F8, tag="whh")
            for g_ in (0, 2, 1):  # r, n, z
                nc.sync.dma_start(out=wih_sb[:, g_ * GW:(g_ + 1) * GW],
                                  in_=wih_d[:, g_ * GW:(g_ + 1) * GW])
                nc.sync.dma_start(out=whh_sb[:, g_ * GW:(g_ + 1) * GW],
                                  in_=whh_d[:, g_ * GW:(g_ + 1) * GW])

            wp_sb = wp_pool.tile([128, DC * VS], F8, tag="wp")
            for b0, b1 in WP_PIECES:
                nc.sync.dma_start(out=wp_sb[:, _bw(b0):_bw(b1)],
                                  in_=wp_d[:, _bw(b0):_bw(b1)])

            logit_sb = big.tile([128, NBLK * B], F16, tag="lg")

            ones_sb = sm.tile([1, 128], F16, tag="ones")
            nc.any.memset(ones_sb[:], 1.0)

            # ---- projection PSUM: 4 banks x 16 block slots of [128, B].
            # One accumulation group per bank may be open at a time (a
            # start pending-zeroes the whole 2KB bank region), so each
            # block's group = bias mm + DC wp mms, sequential per bank;
            # closed blocks' values persist until their flush copy. ----
            ps_tiles = [ps_p.tile([128, 16 * B], F32, tag=f"p{i}",
                                  name=f"pst{i}")
                        for i in range(len(FLUSHES))]

            # one PSUM bank per flush group: tail blocks must not share a
            # bank with an earlier flush's blocks, or the bank-granular
            # WAR against that flush's PSUM->SBUF copy gates the tail mms
            def blk_ps(blk):
                fi = next(i for i, (b0, b1) in enumerate(FLUSHES)
                          if b0 <= blk < b1)
                o = (blk - FLUSHES[fi][0]) * B
                return ps_tiles[fi][:, o:o + B]

            # ==== GRU cell entirely in transposed space: all [d, b]
            # tensors are [128, DC*B] tiles with column = k*B + b. ====

            # x0T = relu(embT rows) -> fp16, one op
            x0t_sb = sm.tile([128, DC * B], F16, tag="x0t")
            nc.scalar.activation(x0t_sb[:], x0f_sb, AF.Relu)

            # bridge, transposed: h0T[d, b] = sum_t hid[t].T @ wbd[t] + bb.
            # hid is fp8 x SH; bb rides a K=1 matmul with a host-side x SH
            # (ones along M, bb_row along N); the copies descale by 1/SH.
            ps_h0 = ps_b.tile([128, DC * B], F32, tag="b")
            for k in range(DC):
                o = ps_h0[:, k * B:(k + 1) * B]
                nc.tensor.matmul(o, ones_sb[:, 0:128],
                                 smalls_sb[:, BB_OFF:BB_OFF + B],
                                 start=True, stop=False)
                for t_i in range(NT):
                    nc.tensor.matmul(
                        o[:, 2 * t_i:2 * t_i + 2],
                        hid_sb[:, t_i * D + k * 128:t_i * D + (k + 1) * 128],
                        wbd_sb[:, t_i * 2:(t_i + 1) * 2],
                        start=False, stop=(t_i == NT - 1),
                    )
            h0t_sb = sm.tile([128, DC * B], F32, tag="h0t")
            nc.scalar.activation(h0t_sb[:], ps_h0[:], AF.Copy, scale=1.0 / SH)
            h0t16 = sm.tile([128, DC * B], F16, tag="h0t16")
            nc.scalar.activation(h0t16[:], ps_h0[:], AF.Copy, scale=1.0 / SH)

            # gates, transposed: gate^T[j, b] accumulated per
            # (j-block jb, k): lhsT = W^T [d-chunk, j-block] (fp8 e3m4,
            # x SG), rhs = x0T / h0T [d-chunk, b] (fp16, true scale).
            # Bias rows (x SG on host) ride K=1 matmuls with values
            # along M; the 1/SG descale hides in the activation scale.
            def gate_psum(g_, use_x, use_h):
                ps = ps_g.tile([128, DC * B], F32, tag="g")
                for jb in range(DC):
                    o = ps[:, jb * B:(jb + 1) * B]
                    bw = BCAT_OFF + g_ * 512 + jb * 128
                    ops = []
                    if use_x:
                        ops.append((smalls_sb[:, bw:bw + 128],
                                    ones_sb[:, 0:B]))
                    if use_h:
                        ops.append((smalls_sb[:, G + bw:G + bw + 128],
                                    ones_sb[:, 0:B]))
                    for k in range(DC):
                        cw = g_ * GW + k * 512 + jb * 128
                        if use_x:
                            ops.append((wih_sb[:, cw:cw + 128],
                                        x0t_sb[:, k * B:(k + 1) * B]))
                        if use_h:
                            ops.append((whh_sb[:, cw:cw + 128],
                                        h0t16[:, k * B:(k + 1) * B]))
                    for i, (lhsT, rhs) in enumerate(ops):
                        nc.tensor.matmul(o, lhsT, rhs, start=(i == 0),
                                         stop=(i == len(ops) - 1))
                return ps

            ps_r = gate_psum(0, True, True)
            ps_xn = gate_psum(2, True, False)
            ps_hn = gate_psum(2, False, True)

            r_sb = sm.tile([128, DC * B], F32, tag="r")
            nc.scalar.activation(r_sb[:], ps_r[:], AF.Sigmoid, scale=1.0 / SG)
            t1 = sm.tile([128, DC * B], F32, tag="t1")
            nc.vector.tensor_mul(t1[:], r_sb[:], ps_hn[:])
            t2 = sm.tile([128, DC * B], F32, tag="t2")
            nc.vector.tensor_add(t2[:], t1[:], ps_xn[:])
            n_sb = sm.tile([128, DC * B], F32, tag="n")
            nc.scalar.activation(n_sb[:], t2[:], AF.Tanh, scale=1.0 / SG)

            ps_z = gate_psum(1, True, True)
            z_sb = sm.tile([128, DC * B], F32, tag="z")
            nc.scalar.activation(z_sb[:], ps_z[:], AF.Sigmoid, scale=1.0 / SG)

            # h1T = n + z * (h0T - n); ship to the projection as
            # h1/SP in fp16 (exact pow2 scaling)
            d_sb = sm.tile([128, DC * B], F32, tag="d")
            nc.vector.tensor_sub(d_sb[:], h0t_sb[:], n_sb[:])
            e_sb = sm.tile([128, DC * B], F32, tag="e")
            nc.vector.tensor_mul(e_sb[:], z_sb[:], d_sb[:])
            h1_sb = sm.tile([128, DC * B], F32, tag="h1")
            nc.vector.tensor_add(h1_sb[:], n_sb[:], e_sb[:])
            h1t_sb = sm.tile([128, DC * B], F16, tag="h1t")
            nc.scalar.activation(h1t_sb[:], h1_sb[:], AF.Copy, scale=1.0 / SP)

            # ---- projection: PSUM[blk][j, b] (+= bp already) +=
            # sum_k wp[blk,k].T @ h1T[k] ----
            flush_at = {b1 - 1: i for i, (b0, b1) in enumerate(FLUSHES)}
            # out-DMA queue per flush: early ones ride gpsimd (Pool idle,
            # off critical path), the tail ones the idle sync queue (the
            # shortest fixed post-wait DGE path of the HWDGE engines)
            F_DMA = [nc.gpsimd, nc.gpsimd, nc.sync, nc.sync]
            for blk in range(NBLK):
                nb = 128 if blk < 49 else LB  # ragged last block
                win = _bw(blk)
                ps = blk_ps(blk)[0:nb, :]
                nc.tensor.matmul(
                    ps,
                    smalls_sb[:, BP_OFF + blk * 128:BP_OFF + blk * 128 + nb],
                    ones_sb[:, 0:B], start=True, stop=False)
                for k in range(DC):
                    nc.tensor.matmul(
                        ps, wp_sb[:, win + k * nb:win + (k + 1) * nb],
                        h1t_sb[:, k * B:(k + 1) * B],
                        start=False, stop=(k == DC - 1))
                if blk in flush_at:
                    fi = flush_at[blk]
                    b0, b1 = FLUSHES[fi]
                    lo, hi = b0 * B, b1 * B
                    nc.scalar.activation(
                        logit_sb[:, lo:hi],
                        ps_tiles[fi][:, 0:(b1 - b0) * B], AF.Copy)
                    F_DMA[fi].dma_start(out=out_d[:, lo:hi],
                                        in_=logit_sb[:, lo:hi])

    nc.compile()
    return nc


def _get_bass():
    if "nc" not in _CACHE:
        _CACHE["nc"] = _build_bass()
    return _CACHE["nc"]


def _interleave(a):
    """[DC*128, N] -> [128, DC*N] with [p, k*N+c] = a[k*128+p, c]."""
    n = a.shape[1]
    return np.ascontiguousarray(
        a.reshape(DC, 128, n).transpose(1, 0, 2).reshape(128, DC * n))


def _prep_inputs(inputs):
    def _e3m4(a):
        # saturate instead of overflowing to inf if a weight ever exceeds
        # the (amply margined) hardcoded pow2 scales
        return np.clip(a, -15.5, 15.5).astype(E3M4)

    ids = np.asarray(inputs["input_ids"])[:, 0].astype(np.int64)
    emb = np.asarray(inputs["emb"], dtype=np.float32)
    hidden = np.asarray(inputs["hidden"], dtype=np.float32)
    Wb = np.asarray(inputs["Wb"], dtype=np.float32)
    bb = np.asarray(inputs["bb"], dtype=np.float32)
    W_ih = np.asarray(inputs["W_ih"], dtype=np.float32)
    b_ih = np.asarray(inputs["b_ih"], dtype=np.float32)
    W_hh = np.asarray(inputs["W_hh"], dtype=np.float32)
    b_hh = np.asarray(inputs["b_hh"], dtype=np.float32)
    Wp = np.asarray(inputs["Wp"], dtype=np.float32)
    bp = np.asarray(inputs["bp"], dtype=np.float32)

    # x0T: [D, B] -> [128, DC*B]
    x0T = _interleave(np.ascontiguousarray(emb[ids].T))
    # hidden rows (b*64+l, d) -> [128, NT*D] with [p, t*D+d] = row t*128+p
    hid8 = _e3m4(np.ascontiguousarray(
        hidden.reshape(NT, 128, D).transpose(1, 0, 2)
        .reshape(128, NT * D) * SH))
    # compact block-diagonal bridge weights: 2 columns per row-tile
    wbd = np.zeros((128, NT * 2), np.float32)
    wbd[0:64, 0::2] = Wb[0][:, None]
    wbd[64:128, 1::2] = Wb[0][:, None]
    head = np.concatenate(
        [wbd.astype(np.float16), x0T.astype(np.float16)], axis=1)

    def _w_layout(w):
        # W [3D, D] -> cols [g, k, j_local], partitions d%128; fp8 x SG
        a = np.ascontiguousarray(w.T * SG).reshape(DC, 128, 3, 512)
        return _e3m4(np.ascontiguousarray(
            a.transpose(1, 2, 0, 3).reshape(128, DC * G)))

    wihT = _w_layout(W_ih)
    whhT = _w_layout(W_hh)

    smalls = np.zeros((1, NSMALL), np.float16)
    smalls[0, BCAT_OFF:BCAT_OFF + 2 * G] = \
        np.concatenate([b_ih, b_hh]) * SG
    smalls[0, BB_OFF:BB_OFF + B] = bb[0] * SH

    # Wp rows padded to VPAD, scaled, fp8; per-core layout
    # wp8[p, blk*512 + k*128 + j] = Wp[c*VS + blk*128 + j, k*128 + p] * SP
    Wp_pad = np.zeros((VPAD, D), np.float32)
    Wp_pad[:V] = Wp * SP
    bp_pad = np.zeros((VPAD,), np.float16)
    bp_pad[:V] = bp

    shared = {"head": head, "hid8": hid8, "wihT": wihT, "whhT": whhT}
    in_maps = []
    for c in range(NCORES):
        m = dict(shared)
        shard = Wp_pad[c * VS:(c + 1) * VS]
        full = shard[:49 * 128].reshape(49, 128, DC, 128) \
            .transpose(3, 0, 2, 1).reshape(128, 49 * 512)
        rag = shard[49 * 128:].reshape(LB, DC, 128) \
            .transpose(2, 1, 0).reshape(128, DC * LB)
        m["wpT"] = _e3m4(np.ascontiguousarray(
            np.concatenate([full, rag], axis=1)))
        sm_c = smalls.copy()
        sm_c[0, BP_OFF:BP_OFF + VS] = bp_pad[c * VS:(c + 1) * VS]
        m["smalls"] = sm_c
        in_maps.append(m)
    return in_maps


def _run(in_maps, trace=False, tmpdir=None):
    nc = _get_bass()
    return run_bass_kernel_spmd(nc, in_maps, list(range(NCORES)),
                                trace=trace, tmpdir=tmpdir)


def kernel(**inputs) -> np.ndarray:
    in_maps = _prep_inputs(inputs)
    try:
        res = _run(in_maps).results
    except Exception:
        # transient NRT device wedges have been observed on this fabric;
        # one retry after a short pause usually lands on healthy cores
        import time as _time
        _time.sleep(5.0)
        res = _run(in_maps).results
    # de-interleave: out[p, blk*B + b] = logits[c*VS + blk*128 + p, b]
    parts = []
    for c in range(NCORES):
        a = np.asarray(res[c]["logits"]).astype(np.float32)
        parts.append(a.reshape(128, NBLK, B).transpose(1, 0, 2)
                     .reshape(NBLK * 128, B)[:VS])
    logits = np.concatenate(parts, axis=0)[:V].T
    logits = np.ascontiguousarray(logits)
    return np.broadcast_to(logits[:, None, :], (B, L - 1, V))
